# revision 1
# baseline (speedup 1.0000x reference)
"""Dense attention kernel for Trainium2, 8 NeuronCores (SPMD).

Problem: q,k,v [8192, 1024] fp32; out = softmax(q @ k.T / sqrt(1024)) @ v.

Strategy (sequence-parallel over q, per the sharding hint):
  - Core c owns q rows [c*1024, (c+1)*1024); k and v are replicated.
  - Host pre-transposes: each core receives qT [D, M]=[1024, 1024] (its q
    shard transposed) and kT [D, N]=[1024, 8192] (k transposed), so the
    contraction dim D is the SBUF partition dim for both matmul operands
    and no on-chip transposes are needed anywhere.
  - Scores are computed TRANSPOSED: sT[n, m] = sum_d kT[d, n] * qT[d, m]
    (lhsT = kT chunk, rhs = qT chunk). The softmax numerator
    pT = exp(sT / 32) then already has the kv dim n on partitions, which is
    exactly the lhsT layout the second matmul needs: o[m, j] += pT.T @ v.
  - No running max: scores/32 ~ N(0,1), max over 8192 ~ 4.3, so exp() is
    bounded by ~e^5 — no overflow risk in fp32, and softmax is shift
    invariant so the result matches the reference.
  - The softmax denominator l[m] = sum_n pT[n, m] falls out of a 1-column
    matmul against a ones vector, accumulated in PSUM alongside o.
  - Final o_acc / l on DVE, then DMA out fp32.

kv is streamed once per core in blocks of NB columns; o/l accumulate in
SBUF fp32 across blocks.
"""

import numpy as np
import ml_dtypes

# ---- problem geometry (hardcoded per contract) ----
N = 8192
D = 1024
NCORES = 8
M = N // NCORES  # 1024 q rows per core

P = 128
DC = D // P  # 8 contraction chunks
NB = 512  # kv block columns
NBLK = N // NB  # 16
NCX = NB // P  # 4 partition-chunks of kv per block
MTS = M // P  # 8 m-tiles per core
MH = 512  # rhs stream width for the scores matmul
NMH = M // MH  # 2

# "bf16": cast q/k/v to bf16 on host, matmuls at full PE rate.
# "f32r": keep fp32 storage, matmuls in float32r (relaxed fp32) mode.
MM_DTYPE = "bf16"

SCALE = 1.0 / np.sqrt(np.float32(D))

_cache = {}


def _build(mm_dtype):
    import concourse.bass as bass
    import concourse.tile as tile
    import concourse.mybir as mybir
    from concourse import bacc

    f32 = mybir.dt.float32
    if mm_dtype == "bf16":
        mdt = mybir.dt.bfloat16
        mmcast = lambda ap: ap
    else:
        mdt = mybir.dt.float32
        mmcast = lambda ap: ap.bitcast(mybir.dt.float32r)

    nc = bacc.Bacc("TRN2", target_bir_lowering=False, debug=False,
                   num_devices=NCORES)
    qT_d = nc.declare_dram_parameter("qT", [D, M], mdt, isOutput=False)
    kT_d = nc.declare_dram_parameter("kT", [D, N], mdt, isOutput=False)
    v_d = nc.declare_dram_parameter("v", [N, D], mdt, isOutput=False)
    o_d = nc.declare_dram_parameter("o", [M, D], f32, isOutput=True)

    kT_r = kT_d.rearrange("(dc p) n -> p dc n", p=P)
    v_r = v_d.rearrange("(nb p) j -> p nb j", p=P)
    o_r = o_d.rearrange("(mt p) j -> p mt j", p=P)

    Exp = mybir.ActivationFunctionType.Exp

    with tile.TileContext(nc) as tc:
        with (
            tc.tile_pool(name="const", bufs=1) as cpool,
            tc.tile_pool(name="qT", bufs=1) as qpool,
            tc.tile_pool(name="acc", bufs=1) as apool,
            tc.tile_pool(name="kT", bufs=3) as kpool,
            tc.tile_pool(name="v", bufs=3) as vpool,
            tc.tile_pool(name="pT", bufs=2) as ppool,
            tc.tile_pool(name="fin", bufs=2) as fpool,
            tc.tile_pool(name="sps", bufs=3, space="PSUM") as spsum,
            tc.tile_pool(name="ops", bufs=2, space="PSUM") as opsum,
            tc.tile_pool(name="lps", bufs=1, space="PSUM") as lpsum,
        ):
            ones = cpool.tile([P, 1], mdt)
            nc.vector.memset(ones[:], 1.0)

            qT_sb = qpool.tile([P, DC, M], mdt)
            nc.sync.dma_start(qT_sb[:], qT_d.rearrange("(dc p) m -> p dc m", p=P))

            o_acc = apool.tile([P, MTS, D], f32)
            l_acc = apool.tile([P, MTS], f32)

            for b in range(NBLK):
                kT_blk = kpool.tile([P, DC, NB], mdt)
                nc.sync.dma_start(kT_blk[:], kT_r[:, :, b * NB:(b + 1) * NB])
                v_blk = vpool.tile([P, NCX, D], mdt)
                nc.sync.dma_start(v_blk[:], v_r[:, b * NCX:(b + 1) * NCX, :])

                pT = ppool.tile([P, NCX, M], mdt)
                for mh in range(NMH):
                    for ncx in range(NCX):
                        sT = spsum.tile([P, MH], f32)
                        for dc in range(DC):
                            nc.tensor.matmul(
                                sT[:],
                                mmcast(kT_blk[:, dc, ncx * P:(ncx + 1) * P]),
                                mmcast(qT_sb[:, dc, mh * MH:(mh + 1) * MH]),
                                start=(dc == 0), stop=(dc == DC - 1),
                            )
                        nc.scalar.activation(
                            pT[:, ncx, mh * MH:(mh + 1) * MH], sT[:],
                            Exp, scale=float(SCALE),
                        )

                l_ps = lpsum.tile([P, MTS], f32)
                for mt in range(MTS):
                    o_ps = opsum.tile([P, D], f32)
                    for ncx in range(NCX):
                        pw = mmcast(pT[:, ncx, mt * P:(mt + 1) * P])
                        nc.tensor.matmul(
                            o_ps[:, 0:512], pw, mmcast(v_blk[:, ncx, 0:512]),
                            start=(ncx == 0), stop=(ncx == NCX - 1),
                        )
                        nc.tensor.matmul(
                            o_ps[:, 512:1024], pw, mmcast(v_blk[:, ncx, 512:1024]),
                            start=(ncx == 0), stop=(ncx == NCX - 1),
                        )
                        nc.tensor.matmul(
                            l_ps[:, mt:mt + 1], pw, mmcast(ones[:]),
                            start=(ncx == 0), stop=(ncx == NCX - 1),
                            skip_group_check=True,
                        )
                    if b == 0:
                        nc.vector.tensor_copy(o_acc[:, mt, :], o_ps[:])
                    else:
                        nc.vector.tensor_add(o_acc[:, mt, :], o_acc[:, mt, :], o_ps[:])
                if b == 0:
                    nc.vector.tensor_copy(l_acc[:], l_ps[:])
                else:
                    nc.vector.tensor_add(l_acc[:], l_acc[:], l_ps[:])

            rcp = fpool.tile([P, MTS], f32, tag="rcp")
            nc.vector.reciprocal(rcp[:], l_acc[:])
            for mt in range(MTS):
                o_out = fpool.tile([P, D], f32, tag="oout")
                nc.vector.tensor_scalar_mul(o_out[:], o_acc[:, mt, :],
                                            rcp[:, mt:mt + 1])
                nc.sync.dma_start(o_r[:, mt, :], o_out[:])

    nc.finalize()
    return nc


def _get_nc():
    if "nc" not in _cache:
        _cache["nc"] = _build(MM_DTYPE)
    return _cache["nc"]


def _prep_inputs(q, k, v):
    npdt = ml_dtypes.bfloat16 if MM_DTYPE == "bf16" else np.float32
    kT = np.ascontiguousarray(np.asarray(k, np.float32).T).astype(npdt)
    vv = np.ascontiguousarray(np.asarray(v, np.float32)).astype(npdt)
    in_maps = []
    for c in range(NCORES):
        qc = np.asarray(q[c * M:(c + 1) * M], np.float32)
        qT = np.ascontiguousarray(qc.T).astype(npdt)
        in_maps.append({"qT": qT, "kT": kT, "v": vv})
    return in_maps


def kernel(q, k, v):
    from concourse.bass_utils import run_bass_kernel_spmd

    nc = _get_nc()
    in_maps = _prep_inputs(q, k, v)
    res = run_bass_kernel_spmd(nc, in_maps, core_ids=list(range(NCORES)))
    return np.concatenate([res.results[c]["o"] for c in range(NCORES)], axis=0)


# revision 4
# speedup vs baseline: 11103.9028x; 11103.9028x over previous
"""Dense attention kernel for Trainium2, 8 NeuronCores (SPMD).

Problem: q,k,v [8192, 1024] fp32; out = softmax(q @ k.T / sqrt(1024)) @ v.

Strategy (sequence-parallel over q, per the sharding hint):
  - Core c owns q rows [c*1024, (c+1)*1024); k and v are replicated.
  - Host pre-transposes: each core receives qT [D, M]=[1024, 1024] (its q
    shard transposed) and kT [D, N]=[1024, 8192] (k transposed), so the
    contraction dim D is the SBUF partition dim for both matmul operands
    and no on-chip transposes are needed anywhere.
  - Scores are computed TRANSPOSED: sT[n, m] = sum_d kT[d, n] * qT[d, m]
    (lhsT = kT chunk, rhs = qT chunk). The softmax numerator
    pT = exp(sT / 32) then already has the kv dim n on partitions, which is
    exactly the lhsT layout the second matmul needs: o[m, j] += pT.T @ v.
  - No running max: scores/32 ~ N(0,1), max over 8192 ~ 4.3, so exp() is
    bounded by ~e^5 — no overflow risk in fp32, and softmax is shift
    invariant so the result matches the reference.
  - The softmax denominator l[m] = sum_n pT[n, m] falls out of a 1-column
    matmul against a ones vector, accumulated in PSUM alongside o.
  - In the last kv block, finalization (l add, reciprocal, scale, store) is
    fused per m-tile so the tail pipelines instead of serializing.

kv is streamed once per core in blocks of NB columns; o/l accumulate in
SBUF fp32 across blocks.

The executor mirrors concourse.bass2jax.run_bass_via_pjrt but caches the
jitted computation (run_bass_via_pjrt re-traces per call). `reps` unrolls
the whole attention pass inside the module for steady-state timing.
"""

import numpy as np
import ml_dtypes

# ---- problem geometry (hardcoded per contract) ----
N = 8192
D = 1024
NCORES = 8
M = N // NCORES  # 1024 q rows per core

P = 128
DC = D // P  # 8 contraction chunks
NB = 512  # kv block columns
NBLK = N // NB  # 16
NCX = NB // P  # 4 partition-chunks of kv per block
MTS = M // P  # 8 m-tiles per core
MH = 512  # rhs stream width for the scores matmul
NMH = M // MH  # 2

# "bf16": cast q/k/v to bf16 on host, matmuls at full PE rate.
# "f32r": keep fp32 storage, matmuls in float32r (relaxed fp32) mode.
MM_DTYPE = "bf16"

SCALE = 1.0 / np.sqrt(np.float32(D))

_cache = {}


def _build(mm_dtype, reps=1):
    import concourse.bass as bass
    import concourse.tile as tile
    import concourse.mybir as mybir
    from concourse import bacc

    f32 = mybir.dt.float32
    if mm_dtype == "bf16":
        mdt = mybir.dt.bfloat16
        mmcast = lambda ap: ap
    else:
        mdt = mybir.dt.float32
        mmcast = lambda ap: ap.bitcast(mybir.dt.float32r)

    nc = bacc.Bacc("TRN2", target_bir_lowering=False, debug=False,
                   num_devices=NCORES)
    qT_d = nc.declare_dram_parameter("qT", [D, M], mdt, isOutput=False)
    kT_d = nc.declare_dram_parameter("kT", [D, N], mdt, isOutput=False)
    v_d = nc.declare_dram_parameter("v", [N, D], mdt, isOutput=False)
    o_d = nc.declare_dram_parameter("o", [M, D], f32, isOutput=True)

    qT_r = qT_d.rearrange("(dc p) m -> p dc m", p=P)
    kT_r = kT_d.rearrange("(dc p) n -> p dc n", p=P)
    v_r = v_d.rearrange("(nb p) j -> p nb j", p=P)
    o_r = o_d.rearrange("(mt p) j -> p mt j", p=P)

    Exp = mybir.ActivationFunctionType.Exp

    # fp32 tiles are 2x the size; shrink buffering to fit SBUF.
    wide = mm_dtype != "bf16"
    qabufs = 1 if (reps == 1 or wide) else 2
    kvbufs = 2 if wide else 3

    with tile.TileContext(nc) as tc:
        with (
            tc.tile_pool(name="const", bufs=1) as cpool,
            tc.tile_pool(name="qT", bufs=qabufs) as qpool,
            tc.tile_pool(name="acc", bufs=qabufs) as apool,
            tc.tile_pool(name="kT", bufs=kvbufs) as kpool,
            tc.tile_pool(name="v", bufs=kvbufs) as vpool,
            tc.tile_pool(name="pT", bufs=2) as ppool,
            tc.tile_pool(name="fin", bufs=2) as fpool,
            tc.tile_pool(name="sps", bufs=3, space="PSUM") as spsum,
            tc.tile_pool(name="ops", bufs=2, space="PSUM") as opsum,
            tc.tile_pool(name="lps", bufs=1, space="PSUM") as lpsum,
        ):
            ones = cpool.tile([P, 1], mdt)
            nc.vector.memset(ones[:], 1.0)

            for _ in range(reps):
                # Per-dc DMA split so the first matmuls start as soon as the
                # first contraction chunks land, not after the whole 2 MB.
                qT_sb = qpool.tile([P, DC, M], mdt)
                for dc in range(DC):
                    nc.sync.dma_start(qT_sb[:, dc, :], qT_r[:, dc, :])

                o_acc = apool.tile([P, MTS, D], f32)
                l_acc = apool.tile([P, MTS], f32)

                for b in range(NBLK):
                    last = b == NBLK - 1
                    kT_blk = kpool.tile([P, DC, NB], mdt)
                    if b == 0:
                        for dc in range(DC):
                            nc.sync.dma_start(kT_blk[:, dc, :],
                                              kT_r[:, dc, b * NB:(b + 1) * NB])
                    else:
                        nc.sync.dma_start(kT_blk[:],
                                          kT_r[:, :, b * NB:(b + 1) * NB])
                    v_blk = vpool.tile([P, NCX, D], mdt)
                    nc.sync.dma_start(v_blk[:], v_r[:, b * NCX:(b + 1) * NCX, :])

                    pT = ppool.tile([P, NCX, M], mdt)
                    for mh in range(NMH):
                        for ncx in range(NCX):
                            sT = spsum.tile([P, MH], f32)
                            for dc in range(DC):
                                nc.tensor.matmul(
                                    sT[:],
                                    mmcast(kT_blk[:, dc, ncx * P:(ncx + 1) * P]),
                                    mmcast(qT_sb[:, dc, mh * MH:(mh + 1) * MH]),
                                    start=(dc == 0), stop=(dc == DC - 1),
                                )
                            nc.scalar.activation(
                                pT[:, ncx, mh * MH:(mh + 1) * MH], sT[:],
                                Exp, scale=float(SCALE),
                            )

                    l_ps = lpsum.tile([P, MTS], f32)
                    for mt in range(MTS):
                        o_ps = opsum.tile([P, D], f32)
                        for ncx in range(NCX):
                            pw = mmcast(pT[:, ncx, mt * P:(mt + 1) * P])
                            nc.tensor.matmul(
                                o_ps[:, 0:512], pw, mmcast(v_blk[:, ncx, 0:512]),
                                start=(ncx == 0), stop=(ncx == NCX - 1),
                            )
                            nc.tensor.matmul(
                                o_ps[:, 512:1024], pw,
                                mmcast(v_blk[:, ncx, 512:1024]),
                                start=(ncx == 0), stop=(ncx == NCX - 1),
                            )
                            nc.tensor.matmul(
                                l_ps[:, mt:mt + 1], pw, mmcast(ones[:]),
                                start=(ncx == 0), stop=(ncx == NCX - 1),
                                skip_group_check=True,
                            )
                        if b == 0:
                            nc.vector.tensor_copy(o_acc[:, mt, :], o_ps[:])
                        elif not last:
                            nc.vector.tensor_add(o_acc[:, mt, :],
                                                 o_acc[:, mt, :], o_ps[:])
                        else:
                            # fused finalization: per-m-tile l total,
                            # reciprocal, o total, scale, store.
                            l_fin = fpool.tile([P, 1], f32, tag="lfin")
                            nc.vector.tensor_add(l_fin[:], l_acc[:, mt:mt + 1],
                                                 l_ps[:, mt:mt + 1])
                            rcp = fpool.tile([P, 1], f32, tag="rcp")
                            nc.vector.reciprocal(rcp[:], l_fin[:])
                            o_fin = fpool.tile([P, D], f32, tag="ofin")
                            nc.vector.tensor_add(o_fin[:], o_acc[:, mt, :],
                                                 o_ps[:])
                            o_out = fpool.tile([P, D], f32, tag="oout")
                            nc.vector.tensor_scalar_mul(o_out[:], o_fin[:],
                                                        rcp[:])
                            nc.sync.dma_start(o_r[:, mt, :], o_out[:])
                    if b == 0:
                        nc.vector.tensor_copy(l_acc[:], l_ps[:])
                    elif not last:
                        nc.vector.tensor_add(l_acc[:], l_acc[:], l_ps[:])

    nc.finalize()
    return nc


def _get_exec(reps=1):
    """Build (once) and cache a jitted SPMD executor whose module runs
    `reps` chained attention passes. Returns (fn, in_names, out_names,
    out_avals); fn(*global_inputs, *global_zero_outs) -> global outputs."""
    key = ("exec", MM_DTYPE, reps)
    if key in _cache:
        return _cache[key]

    import jax
    from jax.sharding import Mesh, PartitionSpec
    from jax.experimental.shard_map import shard_map
    import concourse.mybir as mybir
    from concourse import bass2jax

    nckey = ("nc", MM_DTYPE, reps)
    if nckey not in _cache:
        _cache[nckey] = _build(MM_DTYPE, reps)
    nc = _cache[nckey]

    bass2jax.install_neuronx_cc_hook()

    partition_name = nc.partition_id_tensor.name if nc.partition_id_tensor else None
    in_names, out_names, out_avals = [], [], []
    for alloc in nc.m.functions[0].allocations:
        if not isinstance(alloc, mybir.MemoryLocationSet):
            continue
        name = alloc.memorylocations[0].name
        if alloc.kind == "ExternalInput":
            if name != partition_name:
                in_names.append(name)
        elif alloc.kind == "ExternalOutput":
            out_names.append(name)
            out_avals.append(jax.core.ShapedArray(
                tuple(alloc.tensor_shape), mybir.dt.np(alloc.dtype)))
    n_params = len(in_names)
    n_outs = len(out_names)
    bind_names = tuple(in_names + out_names + (
        [partition_name] if partition_name else []))

    def _body(*args):
        operands = list(args)
        if partition_name is not None:
            operands.append(bass2jax.partition_id_tensor())
        outs = bass2jax._bass_exec_p.bind(
            *operands,
            out_avals=tuple(out_avals),
            in_names=bind_names,
            out_names=tuple(out_names),
            lowering_input_output_aliases=(),
            sim_require_finite=True,
            sim_require_nnan=True,
            nc=nc,
        )
        return tuple(outs)

    devices = jax.devices()[:NCORES]
    mesh = Mesh(np.asarray(devices), ("core",))
    donate = tuple(range(n_params, n_params + n_outs))
    fn = jax.jit(shard_map(
        _body, mesh=mesh,
        in_specs=(PartitionSpec("core"),) * (n_params + n_outs),
        out_specs=(PartitionSpec("core"),) * n_outs,
        check_rep=False,
    ), donate_argnums=donate, keep_unused=True)
    _cache[key] = (fn, in_names, out_names, out_avals)
    return _cache[key]


def _prep_inputs(q, k, v):
    """Per-core host preprocessing -> dict name -> global concat array."""
    npdt = ml_dtypes.bfloat16 if MM_DTYPE == "bf16" else np.float32
    kT = np.ascontiguousarray(np.asarray(k, np.float32).T).astype(npdt)
    vv = np.ascontiguousarray(np.asarray(v, np.float32)).astype(npdt)
    q = np.asarray(q, np.float32)
    qT_g = np.concatenate(
        [np.ascontiguousarray(q[c * M:(c + 1) * M].T).astype(npdt)
         for c in range(NCORES)], axis=0)
    kT_g = np.tile(kT, (NCORES, 1))
    v_g = np.tile(vv, (NCORES, 1))
    return {"qT": qT_g, "kT": kT_g, "v": v_g}


def kernel(q, k, v):
    fn, in_names, out_names, out_avals = _get_exec(reps=1)
    global_ins = _prep_inputs(q, k, v)
    zeros = [np.zeros((NCORES * a.shape[0], *a.shape[1:]), a.dtype)
             for a in out_avals]
    outs = fn(*[global_ins[nm] for nm in in_names], *zeros)
    o = np.asarray(outs[out_names.index("o")])
    return o.reshape(NCORES * M, D)
